# revision 8
# baseline (speedup 1.0000x reference)
"""Level-1 3D Haar DWT (frames, height, width) on video [4,3,16,256,256] f32.

Returns the 8 subbands (aaa, aad, ada, add, daa, dad, dda, ddd), each
[4, 3, 8, 128, 128] f32, matching pywt level-1 wavedec3 conventions:
cA = (x0 + x1)/sqrt2, cD = (x0 - x1)/sqrt2 applied separably over
frames (axis 2), height (axis 3), width (axis 4).

Distribution: pure data parallel over the 8 frame-pairs. Core k
processes video[:, :, 2k:2k+2, :, :] -> [12, 2, 256, 256]; every
(b, c, frame-pair) is independent, so there is no communication.

Per-core kernel (Bass/Tile), per chunk of pairs:
  - load:    one HWDGE DMA; x viewed [pair, h, frame, r*256+w] so the
             chunk is a 3-dim access pattern (partition = row-pair h).
  - scale:   T *= 2^-1.5 on the otherwise-idle ScalarE (folds all three
             1/sqrt2 stage scales, keeps DVE/GPSIMD ops plain add/sub).
  - F stage  (DVE):    A = f0 + f1, D = f0 - f1
  - H stage  (GPSIMD): AA = Ae + Ao, AD = Ae - Ao, ... (offloads the DVE)
  - W stage  (DVE):    even/odd column sums/diffs into subband-major Y.
  - store:   one HWDGE DMA per pair (dst runs 4KB contiguous/partition).
"""

import math

import numpy as np

import concourse.bass as bass
import concourse.bacc as bacc
import concourse.mybir as mybir
from concourse.bass_utils import run_bass_kernel_spmd
from concourse.tile import TileContext

F32 = mybir.dt.float32
NCORES = 8
NPAIRS = 12          # B*C = 4*3 (b,c) slices per core, one frame-pair each
CHUNK = 3            # pairs per pipeline chunk
C3 = (1.0 / math.sqrt(2.0)) ** 3   # scale for all three stages

_CACHE = {}


def _build_bass():
    nc = bacc.Bacc()
    x = nc.dram_tensor("x", [NPAIRS, 128, 2, 512], F32, kind="ExternalInput")
    y = nc.dram_tensor("y", [NPAIRS, 128, 1024], F32, kind="ExternalOutput")

    add = mybir.AluOpType.add
    sub = mybir.AluOpType.subtract

    with TileContext(nc) as tc:
        with tc.tile_pool(name="io", bufs=2) as io_pool, \
             tc.tile_pool(name="mid", bufs=2) as mid_pool:
            for ci in range(NPAIRS // CHUNK):
                p0 = ci * CHUNK
                # ---- load: x[CHUNK, 128, 2, 512] -> T[h, p, f, rw] ----
                T = io_pool.tile([128, CHUNK, 2, 512], F32, name="T")
                nc.sync.dma_start(
                    out=T[:, :, :, :],
                    in_=x[p0:p0 + CHUNK].rearrange("p h f rw -> h p (f rw)", f=2),
                )

                # ---- scale by 2^-1.5 on ScalarE (otherwise idle) ----
                nc.scalar.mul(T.rearrange("h p f rw -> h (p f rw)"),
                              T.rearrange("h p f rw -> h (p f rw)"), C3)

                # ---- F stage (frames): A = f0+f1, D = f0-f1 (DVE) ----
                A = mid_pool.tile([128, CHUNK, 2, 256], F32, name="A")
                D = mid_pool.tile([128, CHUNK, 2, 256], F32, name="D")
                f0 = T[:, :, 0, :]
                f1 = T[:, :, 1, :]
                Av = A.rearrange("h p r w -> h p (r w)")
                Dv = D.rearrange("h p r w -> h p (r w)")
                nc.vector.tensor_tensor(Av, f0, f1, add)
                nc.vector.tensor_tensor(Dv, f0, f1, sub)

                # ---- H stage (rows): even row +/- odd row (GPSIMD) ----
                AA = mid_pool.tile([128, CHUNK, 256], F32, name="AA")
                AD = mid_pool.tile([128, CHUNK, 256], F32, name="AD")
                DA = mid_pool.tile([128, CHUNK, 256], F32, name="DA")
                DD = mid_pool.tile([128, CHUNK, 256], F32, name="DD")
                nc.gpsimd.tensor_tensor(AA[:, :, :], A[:, :, 0, :], A[:, :, 1, :], add)
                nc.gpsimd.tensor_tensor(AD[:, :, :], A[:, :, 0, :], A[:, :, 1, :], sub)
                nc.gpsimd.tensor_tensor(DA[:, :, :], D[:, :, 0, :], D[:, :, 1, :], add)
                nc.gpsimd.tensor_tensor(DD[:, :, :], D[:, :, 0, :], D[:, :, 1, :], sub)

                # ---- W stage (cols): even col +/- odd col (DVE) ----
                Y = io_pool.tile([128, 8, CHUNK, 128], F32, name="Y")
                for s, src in enumerate((AA, AD, DA, DD)):
                    v = src.rearrange("h p (w r) -> h (p w) r", r=2)
                    xe = v[:, :, 0]
                    xo = v[:, :, 1]
                    ye = Y[:, 2 * s].rearrange("h p w -> h (p w)")
                    yo = Y[:, 2 * s + 1].rearrange("h p w -> h (p w)")
                    nc.vector.tensor_tensor(ye, xe, xo, add)
                    nc.vector.tensor_tensor(yo, xe, xo, sub)

                # ---- store: Y[h, s, p, w] -> y[p, h, s*128+w] ----
                for p in range(CHUNK):
                    nc.sync.dma_start(
                        out=y[p0 + p].rearrange("h (s w) -> h s w", s=8),
                        in_=Y[:, :, p, :],
                    )
    # Bacc post-passes legalize multi-wait instructions (event semaphores)
    # and populate InstISA instr bytes — without this walrus rejects the
    # program ("Too many sync wait commands" / "ISA wrong length").
    nc.compile()
    return nc


def _get_nc():
    if "nc" not in _CACHE:
        _CACHE["nc"] = _build_bass()
    return _CACHE["nc"]


def _shard_inputs(video):
    video = np.ascontiguousarray(np.asarray(video), dtype=np.float32)
    in_maps = []
    for k in range(NCORES):
        shard = video[:, :, 2 * k:2 * k + 2]                     # [4,3,2,256,256]
        shard = shard.reshape(NPAIRS, 2, 128, 512)               # (p, f, h, rw)
        shard = np.ascontiguousarray(shard.transpose(0, 2, 1, 3))  # (p, h, f, rw)
        in_maps.append({"x": shard})
    return in_maps


def _unshard_outputs(results):
    # results[k]["y"]: [12, 128, 1024]; dims (p, h, (s, w)); frame index = k
    ys = np.stack([np.asarray(r["y"]) for r in results])        # [8,12,128,1024]
    ys = ys.reshape(NCORES, 4, 3, 128, 8, 128)                  # (f,b,c,h,s,w)
    ys = ys.transpose(4, 1, 2, 0, 3, 5)                         # (s,b,c,f,h,w)
    return tuple(np.ascontiguousarray(ys[s]) for s in range(8))


def run(video, **spmd_kwargs):
    nc = _get_nc()
    res = run_bass_kernel_spmd(
        nc, _shard_inputs(video), core_ids=list(range(NCORES)), **spmd_kwargs
    )
    return _unshard_outputs(res.results), res


def kernel(video):
    out, _ = run(video)
    return out


# revision 9
# speedup vs baseline: 1.2521x; 1.2521x over previous
"""Level-1 3D Haar DWT (frames, height, width) on video [4,3,16,256,256] f32.

Returns the 8 subbands (aaa, aad, ada, add, daa, dad, dda, ddd), each
[4, 3, 8, 128, 128] f32, matching pywt level-1 wavedec3 conventions:
cA = (x0 + x1)/sqrt2, cD = (x0 - x1)/sqrt2 applied separably over
frames (axis 2), height (axis 3), width (axis 4).

Distribution: pure data parallel over the 8 frame-pairs. Core k
processes video[:, :, 2k:2k+2, :, :] -> [12, 2, 256, 256]; every
(b, c, frame-pair) is independent, so there is no communication.

Per-core kernel (Bass/Tile), per chunk of CHUNK pairs:
  - load:  one HWDGE DMA; x viewed [pair, h, frame, r*256+w], so the
           chunk is a 3-dim access pattern (partition = row-pair h).
  - F stage (DVE tensor_tensor): A = f0 + f1, D = f0 - f1.
  - H stage (DVE tensor_tensor): AA = Ae + Ao, AD = Ae - Ao, ...
  - W stage (DVE ln_bwd_dx):     (xe -/+ xo) * 2^-1.5 fused — all three
           1/sqrt2 stage scales ride the last op for free.
  All compute stays on the DVE: GpSimd shares (and exclusively locks)
  the DVE's second SBUF read port, so offloading tensor_tensor to it
  only serializes; ACT/PE can't do 2-input fp32 elementwise at rate.
  - store: one HWDGE DMA per chunk into an h-major DRAM layout
           y[h, s, pair, w] (>=2KB contiguous runs on the HBM side).
"""

import math

import numpy as np

import concourse.bacc as bacc
import concourse.mybir as mybir
from concourse.bass_utils import run_bass_kernel_spmd
from concourse.tile import TileContext

F32 = mybir.dt.float32
NCORES = 8
NPAIRS = 12          # B*C = 4*3 (b,c) slices per core, one frame-pair each
CHUNK = 4            # pairs per pipeline chunk
C3 = (1.0 / math.sqrt(2.0)) ** 3   # scale for all three stages

_CACHE = {}


def _build_bass():
    nc = bacc.Bacc()
    x = nc.dram_tensor("x", [NPAIRS, 128, 2, 512], F32, kind="ExternalInput")
    # h-major output: y[h, s, p, w]
    y = nc.dram_tensor("y", [128, 8, NPAIRS, 128], F32, kind="ExternalOutput")

    add = mybir.AluOpType.add
    sub = mybir.AluOpType.subtract

    with TileContext(nc) as tc:
        with tc.tile_pool(name="io", bufs=2) as io_pool, \
             tc.tile_pool(name="mid", bufs=2) as mid_pool:
            for ci in range(NPAIRS // CHUNK):
                p0 = ci * CHUNK
                # ---- load: x[CHUNK, 128, 2, 512] -> T[h, p, f, rw] ----
                T = io_pool.tile([128, CHUNK, 2, 512], F32, name="T")
                nc.sync.dma_start(
                    out=T[:, :, :, :],
                    in_=x[p0:p0 + CHUNK].rearrange("p h f rw -> h p (f rw)", f=2),
                )

                # ---- F stage (frames): A = f0+f1, D = f0-f1 ----
                A = mid_pool.tile([128, CHUNK, 2, 256], F32, name="A")
                D = mid_pool.tile([128, CHUNK, 2, 256], F32, name="D")
                f0 = T[:, :, 0, :]
                f1 = T[:, :, 1, :]
                Av = A.rearrange("h p r w -> h p (r w)")
                Dv = D.rearrange("h p r w -> h p (r w)")
                nc.vector.tensor_tensor(Av, f0, f1, add)
                nc.vector.tensor_tensor(Dv, f0, f1, sub)

                # ---- H stage (rows): even row +/- odd row ----
                AA = mid_pool.tile([128, CHUNK, 256], F32, name="AA")
                AD = mid_pool.tile([128, CHUNK, 256], F32, name="AD")
                DA = mid_pool.tile([128, CHUNK, 256], F32, name="DA")
                DD = mid_pool.tile([128, CHUNK, 256], F32, name="DD")
                nc.vector.tensor_tensor(AA[:, :, :], A[:, :, 0, :], A[:, :, 1, :], add)
                nc.vector.tensor_tensor(AD[:, :, :], A[:, :, 0, :], A[:, :, 1, :], sub)
                nc.vector.tensor_tensor(DA[:, :, :], D[:, :, 0, :], D[:, :, 1, :], add)
                nc.vector.tensor_tensor(DD[:, :, :], D[:, :, 0, :], D[:, :, 1, :], sub)

                # ---- W stage (cols): (xe +/- xo) * C3, fused scale ----
                Y = io_pool.tile([128, 8, CHUNK, 128], F32, name="Y")
                for s, src in enumerate((AA, AD, DA, DD)):
                    v = src.rearrange("h p (w r) -> h (p w) r", r=2)
                    xe = v[:, :, 0]
                    xo = v[:, :, 1]
                    ye = Y[:, 2 * s].rearrange("h p w -> h (p w)")
                    yo = Y[:, 2 * s + 1].rearrange("h p w -> h (p w)")
                    # ln_bwd_dx: out = (in0 - in1*s0 - s1) * imm2
                    nc.vector.ln_bwd_dx(ye, xe, xo, -1.0, 0.0, C3)
                    nc.vector.ln_bwd_dx(yo, xe, xo, 1.0, 0.0, C3)

                # ---- store: Y[h, s, p, w] -> y[h, s, p0:p0+CHUNK, w] ----
                nc.sync.dma_start(
                    out=y[:, :, p0:p0 + CHUNK].rearrange("h s p w -> h s (p w)"),
                    in_=Y.rearrange("h s p w -> h s (p w)"),
                )
    # Bacc post-passes legalize multi-wait instructions (event semaphores)
    # and populate InstISA instr bytes for the custom DVE op.
    nc.compile()
    return nc


def _get_nc():
    if "nc" not in _CACHE:
        _CACHE["nc"] = _build_bass()
    return _CACHE["nc"]


def _shard_inputs(video):
    video = np.ascontiguousarray(np.asarray(video), dtype=np.float32)
    in_maps = []
    for k in range(NCORES):
        shard = video[:, :, 2 * k:2 * k + 2]                     # [4,3,2,256,256]
        shard = shard.reshape(NPAIRS, 2, 128, 512)               # (p, f, h, rw)
        shard = np.ascontiguousarray(shard.transpose(0, 2, 1, 3))  # (p, h, f, rw)
        in_maps.append({"x": shard})
    return in_maps


def _unshard_outputs(results):
    # results[k]["y"]: [128, 8, 12, 128] dims (h, s, p, w); frame index = k
    ys = np.stack([np.asarray(r["y"]) for r in results])        # [8,128,8,12,128]
    ys = ys.reshape(NCORES, 128, 8, 4, 3, 128)                  # (f,h,s,b,c,w)
    ys = ys.transpose(2, 3, 4, 0, 1, 5)                         # (s,b,c,f,h,w)
    return tuple(np.ascontiguousarray(ys[s]) for s in range(8))


def run(video, **spmd_kwargs):
    nc = _get_nc()
    res = run_bass_kernel_spmd(
        nc, _shard_inputs(video), core_ids=list(range(NCORES)), **spmd_kwargs
    )
    return _unshard_outputs(res.results), res


def kernel(video):
    out, _ = run(video)
    return out


# revision 10
# speedup vs baseline: 1.3343x; 1.0656x over previous
"""Level-1 3D Haar DWT (frames, height, width) on video [4,3,16,256,256] f32.

Returns the 8 subbands (aaa, aad, ada, add, daa, dad, dda, ddd), each
[4, 3, 8, 128, 128] f32, matching pywt level-1 wavedec3 conventions:
cA = (x0 + x1)/sqrt2, cD = (x0 - x1)/sqrt2 applied separably over
frames (axis 2), height (axis 3), width (axis 4).

Distribution: pure data parallel over the 8 frame-pairs. Core k
processes video[:, :, 2k:2k+2, :, :] -> [12, 2, 256, 256]; every
(b, c, frame-pair) is independent, so there is no communication.

Per-core kernel (Bass/Tile), per chunk of pairs (ragged [5, 5, 2]):
  - load:  HWDGE DMA on the ACT ring (per pair in chunk 0 so the DVE can
           start after ~1.5us instead of waiting for a full-chunk load);
           x viewed [pair, h, frame, r*256+w] (partition = row-pair h).
  - F stage (DVE tensor_tensor): A = f0 + f1, D = f0 - f1.
  - H stage (DVE tensor_tensor): AA = Ae + Ao, AD = Ae - Ao, ...
  - W stage (DVE ln_bwd_dx):     (xe -/+ xo) * 2^-1.5 fused — all three
           1/sqrt2 stage scales ride the last op for free.
  All compute stays on the DVE: GpSimd shares (and exclusively locks)
  the DVE's second SBUF read port, so offloading tensor_tensor to it
  only serializes; ACT/PE can't do 2-input fp32 elementwise at rate.
  - store: one HWDGE DMA per chunk (SP ring) into an h-major DRAM
           layout y[h, s, pair, w] (2KB+ contiguous runs on HBM).
"""

import math

import numpy as np

import concourse.bacc as bacc
import concourse.mybir as mybir
from concourse.bass_utils import run_bass_kernel_spmd
from concourse.tile import TileContext

F32 = mybir.dt.float32
NCORES = 8
NPAIRS = 12          # B*C = 4*3 (b,c) slices per core, one frame-pair each
CHUNKS = (5, 5, 2)   # ragged: small last chunk shortens the store tail
C3 = (1.0 / math.sqrt(2.0)) ** 3   # scale for all three stages

_CACHE = {}


def _build_bass():
    nc = bacc.Bacc()
    x = nc.dram_tensor("x", [NPAIRS, 128, 2, 512], F32, kind="ExternalInput")
    # h-major output: y[h, s, p, w]
    y = nc.dram_tensor("y", [128, 8, NPAIRS, 128], F32, kind="ExternalOutput")

    add = mybir.AluOpType.add
    sub = mybir.AluOpType.subtract

    with TileContext(nc) as tc:
        with tc.tile_pool(name="io", bufs=2) as io_pool, \
             tc.tile_pool(name="mid", bufs=2) as mid_pool:
            p0 = 0
            for ci, ch in enumerate(CHUNKS):
                # ---- load: x[ch, 128, 2, 512] -> T[h, p, f, rw] ----
                # ACT HWDGE ring, so stores on the SP ring can't queue ahead
                T = io_pool.tile([128, ch, 2, 512], F32, name="T", tag="T",
                                 padded_shape=[128, max(CHUNKS), 2, 512])
                if ci == 0:
                    for p in range(ch):   # per-pair: DVE starts ~4us earlier
                        nc.scalar.dma_start(
                            out=T[:, p],
                            in_=x[p0 + p].rearrange("h f rw -> h (f rw)", f=2),
                        )
                else:
                    nc.scalar.dma_start(
                        out=T[:, :, :, :],
                        in_=x[p0:p0 + ch].rearrange("p h f rw -> h p (f rw)", f=2),
                    )

                # ---- F stage (frames): A = f0+f1, D = f0-f1 ----
                A = mid_pool.tile([128, ch, 2, 256], F32, name="A", tag="A",
                                  padded_shape=[128, max(CHUNKS), 2, 256])
                D = mid_pool.tile([128, ch, 2, 256], F32, name="D", tag="D",
                                  padded_shape=[128, max(CHUNKS), 2, 256])
                if ci == 0:
                    for p in range(ch):   # per-pair, tracks the per-pair loads
                        nc.vector.tensor_tensor(
                            A[:, p].rearrange("h r w -> h (r w)"),
                            T[:, p, 0, :], T[:, p, 1, :], add)
                        nc.vector.tensor_tensor(
                            D[:, p].rearrange("h r w -> h (r w)"),
                            T[:, p, 0, :], T[:, p, 1, :], sub)
                else:
                    Av = A.rearrange("h p r w -> h p (r w)")
                    Dv = D.rearrange("h p r w -> h p (r w)")
                    nc.vector.tensor_tensor(Av, T[:, :, 0, :], T[:, :, 1, :], add)
                    nc.vector.tensor_tensor(Dv, T[:, :, 0, :], T[:, :, 1, :], sub)

                # ---- H stage (rows): even row +/- odd row ----
                AA = mid_pool.tile([128, ch, 256], F32, name="AA", tag="AA",
                                   padded_shape=[128, max(CHUNKS), 256])
                AD = mid_pool.tile([128, ch, 256], F32, name="AD", tag="AD",
                                   padded_shape=[128, max(CHUNKS), 256])
                DA = mid_pool.tile([128, ch, 256], F32, name="DA", tag="DA",
                                   padded_shape=[128, max(CHUNKS), 256])
                DD = mid_pool.tile([128, ch, 256], F32, name="DD", tag="DD",
                                   padded_shape=[128, max(CHUNKS), 256])
                nc.vector.tensor_tensor(AA[:, :, :], A[:, :, 0, :], A[:, :, 1, :], add)
                nc.vector.tensor_tensor(AD[:, :, :], A[:, :, 0, :], A[:, :, 1, :], sub)
                nc.vector.tensor_tensor(DA[:, :, :], D[:, :, 0, :], D[:, :, 1, :], add)
                nc.vector.tensor_tensor(DD[:, :, :], D[:, :, 0, :], D[:, :, 1, :], sub)

                # ---- W stage (cols): (xe +/- xo) * C3, fused scale ----
                Y = io_pool.tile([128, 8, ch, 128], F32, name="Y", tag="Y",
                                 padded_shape=[128, 8, max(CHUNKS), 128])
                for s, src in enumerate((AA, AD, DA, DD)):
                    v = src.rearrange("h p (w r) -> h (p w) r", r=2)
                    xe = v[:, :, 0]
                    xo = v[:, :, 1]
                    ye = Y[:, 2 * s].rearrange("h p w -> h (p w)")
                    yo = Y[:, 2 * s + 1].rearrange("h p w -> h (p w)")
                    # ln_bwd_dx: out = (in0 - in1*s0 - s1) * imm2
                    nc.vector.ln_bwd_dx(ye, xe, xo, -1.0, 0.0, C3)
                    nc.vector.ln_bwd_dx(yo, xe, xo, 1.0, 0.0, C3)

                # ---- store: Y[h, s, p, w] -> y[h, s, p0:p0+ch, w] ----
                nc.sync.dma_start(
                    out=y[:, :, p0:p0 + ch].rearrange("h s p w -> h s (p w)"),
                    in_=Y.rearrange("h s p w -> h s (p w)"),
                )
                p0 += ch
    # Bacc post-passes legalize multi-wait instructions (event semaphores)
    # and populate InstISA instr bytes for the custom DVE op.
    nc.compile()
    return nc


def _get_nc():
    if "nc" not in _CACHE:
        _CACHE["nc"] = _build_bass()
    return _CACHE["nc"]


def _shard_inputs(video):
    video = np.ascontiguousarray(np.asarray(video), dtype=np.float32)
    in_maps = []
    for k in range(NCORES):
        shard = video[:, :, 2 * k:2 * k + 2]                     # [4,3,2,256,256]
        shard = shard.reshape(NPAIRS, 2, 128, 512)               # (p, f, h, rw)
        shard = np.ascontiguousarray(shard.transpose(0, 2, 1, 3))  # (p, h, f, rw)
        in_maps.append({"x": shard})
    return in_maps


def _unshard_outputs(results):
    # results[k]["y"]: [128, 8, 12, 128] dims (h, s, p, w); frame index = k
    ys = np.stack([np.asarray(r["y"]) for r in results])        # [8,128,8,12,128]
    ys = ys.reshape(NCORES, 128, 8, 4, 3, 128)                  # (f,h,s,b,c,w)
    ys = ys.transpose(2, 3, 4, 0, 1, 5)                         # (s,b,c,f,h,w)
    return tuple(np.ascontiguousarray(ys[s]) for s in range(8))


def run(video, **spmd_kwargs):
    nc = _get_nc()
    res = run_bass_kernel_spmd(
        nc, _shard_inputs(video), core_ids=list(range(NCORES)), **spmd_kwargs
    )
    return _unshard_outputs(res.results), res


def kernel(video):
    out, _ = run(video)
    return out


# revision 11
# speedup vs baseline: 1.5487x; 1.1607x over previous
"""Level-1 3D Haar DWT on video [4,3,16,256,256] f32 -> 8 subbands
[4,3,8,128,128], pywt convention (cA=(x0+x1)/sqrt2, cD=(x0-x1)/sqrt2 over
frames, height, width).

Distribution: pure data parallel over the 8 frame pairs (F=16 -> 8
independent pairs); core k processes video[:, :, 2k:2k+2] with zero
cross-core communication.

Per-core pipeline (Bass/Tile), ragged chunks of CH pairs, row-half
u in {0,1} (rows u*128..u*128+127 of each frame):
  load (sync HWDGE ring): X[f,u] = x[p, f, u-half]  [128 rows, CH, 256]
  F stage (DVE):  A_u = X[0,u] + X[1,u]; D_u = X[0,u] - X[1,u]
  H stage (PE):   P_t_u = B.T @ (A|D)_u -> PSUM, B (+-1, fp32-exact):
                  out[j] = in[2j] + in[2j+1]      (aa rows 0..63)
                  out[64+j] = in[2j] - in[2j+1]   (ad rows 64..127)
                  TensorE has its own SBUF ports, so this runs fully
                  parallel to the DVE (GpSimd would lock the DVE's
                  second read port instead - measured, not theoretical).
  evac (ACT):     odd columns of P -> SBUF (a 2-input DVE op may read
                  at most one operand from PSUM)
  W stage (DVE):  (xe -/+ xo) * 2^-1.5 via the fused LN_BWD_DX custom
                  op; xe strided from PSUM, xo from SBUF; all three
                  1/sqrt2 stage scales folded here.
  store (scalar HWDGE ring): h-major DRAM layout, 512B+ runs.

Output DRAM y[u, j, t, e, p, w]: subband s = (t, j>=64, e), h = u*64+j%64.
"""

import math

import numpy as np

import concourse.bacc as bacc
import concourse.mybir as mybir
from concourse.bass_utils import run_bass_kernel_spmd
from concourse.tile import TileContext

F32 = mybir.dt.float32
NCORES = 8
NPAIRS = 12
CHUNKS = (2, 4, 4, 2)   # ragged: short first/last chunks trim fill/drain
CHMAX = max(CHUNKS)
NCHUNK = len(CHUNKS)
C3 = (1.0 / math.sqrt(2.0)) ** 3

_CACHE = {}


def _bmat():
    b = np.zeros((128, 128), np.float32)
    for j in range(64):
        b[2 * j, j] = 1.0
        b[2 * j + 1, j] = 1.0
        b[2 * j, 64 + j] = 1.0
        b[2 * j + 1, 64 + j] = -1.0
    return b


def _build_bass():
    nc = bacc.Bacc()
    x = nc.dram_tensor("x", [NPAIRS, 2, 256, 256], F32, kind="ExternalInput")
    bm = nc.dram_tensor("bmat", [128, 128], F32, kind="ExternalInput")
    y = nc.dram_tensor("y", [2, 128, 2, 2, NPAIRS, 128], F32,
                       kind="ExternalOutput")

    add = mybir.AluOpType.add
    sub = mybir.AluOpType.subtract

    with TileContext(nc) as tc:
        with tc.tile_pool(name="const", bufs=1) as cpool, \
             tc.tile_pool(name="io", bufs=3) as io_pool, \
             tc.tile_pool(name="mid", bufs=3) as mid_pool, \
             tc.tile_pool(name="ps", bufs=1, space="PSUM") as ps_pool:
            B = cpool.tile([128, 128], F32, name="B")
            nc.sync.dma_start(out=B[:, :], in_=bm[:, :])
            p0 = 0
            for ci, CH in enumerate(CHUNKS):
                X = {}
                for u in range(2):
                    for f in range(2):
                        Xt = io_pool.tile([128, CH, 256], F32, name="X",
                                          tag=f"X{f}{u}",
                                          padded_shape=[128, CHMAX, 256])
                        nc.sync.dma_start(
                            out=Xt[:, :, :],
                            in_=x[p0:p0 + CH, f, 128 * u:128 * (u + 1)]
                                .rearrange("p r w -> r p w"),
                        )
                        X[(f, u)] = Xt
                # F stage: A_u = f0 + f1, D_u = f0 - f1
                AD = {}
                for u in range(2):
                    for t in range(2):       # 0: A (sum), 1: D (diff)
                        M = mid_pool.tile([128, CH, 256], F32, name="M",
                                          tag=f"M{t}{u}",
                                          padded_shape=[128, CHMAX, 256])
                        nc.vector.tensor_tensor(
                            M[:, :, :], X[(0, u)][:, :, :], X[(1, u)][:, :, :],
                            add if t == 0 else sub)
                        AD[(t, u)] = M
                # H stage on PE -> PSUM
                E = {}
                for (t, u), M in AD.items():
                    i = 2 * t + u
                    P = ps_pool.tile([128, CH, 256], F32, name="P", tag=f"P{i}",
                                     padded_shape=[128, CHMAX, 256])
                    Pf = P.rearrange("j p w -> j (p w)")
                    Mf = M.rearrange("j p w -> j (p w)")
                    for n0 in range(0, CH * 256, 512):  # one PSUM bank per matmul
                        n1 = min(n0 + 512, CH * 256)
                        nc.tensor.matmul(
                            Pf[:, n0:n1], B[:, :], Mf[:, n0:n1])
                    # evacuate only the odd columns (ACT): the W-stage
                    # 2-input op may read at most one operand from PSUM
                    Od = mid_pool.tile([128, CH * 128], F32, name="Od",
                                       tag=f"O{i}",
                                       padded_shape=[128, CHMAX * 128])
                    nc.scalar.copy(
                        Od[:, :],
                        P.rearrange("j p (w r) -> j (p w) r", r=2)[:, :, 1])
                    E[(t, u)] = (P, Od)
                # W stage (DVE): even cols from PSUM, odd from SBUF, *C3 fused
                for u in range(2):
                    YU = io_pool.tile([128, 2, 2, CH * 128], F32, name="YU",
                                      tag=f"YU{u}",
                                      padded_shape=[128, 2, 2, CHMAX * 128])
                    for t in range(2):
                        P, Od = E[(t, u)]
                        xe = P.rearrange("j p (w r) -> j (p w) r", r=2)[:, :, 0]
                        xo = Od[:, :]
                        # out = (in0 - in1*s0 - s1) * imm2
                        nc.vector.ln_bwd_dx(YU[:, t, 0, :], xe, xo, -1.0, 0.0, C3)
                        nc.vector.ln_bwd_dx(YU[:, t, 1, :], xe, xo, 1.0, 0.0, C3)
                    nc.scalar.dma_start(
                        out=y[u, :, :, :, p0:p0 + CH]
                            .rearrange("j t e p w -> j t e (p w)"),
                        in_=YU[:, :, :, :],
                    )
                p0 += CH
    nc.compile()
    return nc


def _get_nc():
    if "nc" not in _CACHE:
        _CACHE["nc"] = _build_bass()
    return _CACHE["nc"]


def _shard_inputs(video):
    video = np.ascontiguousarray(np.asarray(video), dtype=np.float32)
    bm = _bmat()
    in_maps = []
    for k in range(NCORES):
        shard = np.ascontiguousarray(
            video[:, :, 2 * k:2 * k + 2]).reshape(NPAIRS, 2, 256, 256)
        in_maps.append({"x": shard, "bmat": bm})
    return in_maps


def _unshard_outputs(results):
    # y[u, j, t, e, p, w]; j = qq*64 + jj; h = u*64 + jj; s = (t, qq, e)
    ys = np.stack([np.asarray(r["y"]) for r in results])  # [8,2,128,2,2,12,128]
    ys = ys.reshape(NCORES, 2, 2, 64, 2, 2, NPAIRS, 128)
    #      dims: (f, u, qq, jj, t, e, p, w)
    ys = ys.transpose(4, 2, 5, 6, 0, 1, 3, 7)
    #      -> (t, qq, e, p, f, u, jj, w)
    ys = ys.reshape(8, 4, 3, NCORES, 128, 128)            # (s, b, c, f, h, w)
    return tuple(np.ascontiguousarray(ys[s]) for s in range(8))


def run(video, **spmd_kwargs):
    nc = _get_nc()
    res = run_bass_kernel_spmd(
        nc, _shard_inputs(video), core_ids=list(range(NCORES)), **spmd_kwargs
    )
    return _unshard_outputs(res.results), res


def kernel(video):
    out, _ = run(video)
    return out


# revision 12
# speedup vs baseline: 1.5933x; 1.0288x over previous
"""Level-1 3D Haar DWT on video [4,3,16,256,256] f32 -> 8 subbands
[4,3,8,128,128], pywt convention (cA=(x0+x1)/sqrt2, cD=(x0-x1)/sqrt2 over
frames, height, width).

Distribution: pure data parallel over the 8 frame pairs (F=16 -> 8
independent pairs); core k processes video[:, :, 2k:2k+2] with zero
cross-core communication.

Per-core pipeline (Bass/Tile), ragged chunks of CH pairs, row-half
u in {0,1} (rows u*128..u*128+127 of each frame):
  load (sync HWDGE ring): X[f,u] = x[p, f, u-half]  [128 rows, CH, 256]
  F stage (DVE):  A_u = X[0,u] + X[1,u]; D_u = X[0,u] - X[1,u]
  H stage (PE):   P_t_u = B.T @ (A|D)_u -> PSUM, B (+-1, fp32-exact):
                  out[j] = in[2j] + in[2j+1]      (aa rows 0..63)
                  out[64+j] = in[2j] - in[2j+1]   (ad rows 64..127)
                  TensorE has its own SBUF ports, so this runs fully
                  parallel to the DVE (GpSimd would lock the DVE's
                  second read port instead - measured, not theoretical).
  evac (ACT):     odd columns of P -> SBUF (a 2-input DVE op may read
                  at most one operand from PSUM)
  W stage (DVE):  (xe -/+ xo) * 2^-1.5 via the fused LN_BWD_DX custom
                  op; xe strided from PSUM, xo from SBUF; all three
                  1/sqrt2 stage scales folded here.
  store (scalar HWDGE ring): h-major DRAM layout, 512B+ runs.

Output DRAM y[u, j, t, e, p, w]: subband s = (t, j>=64, e), h = u*64+j%64.
"""

import math

import numpy as np

import concourse.bacc as bacc
import concourse.mybir as mybir
from concourse.bass_utils import run_bass_kernel_spmd
from concourse.tile import TileContext

F32 = mybir.dt.float32
NCORES = 8
NPAIRS = 12
CHUNKS = (2, 4, 4, 2)   # ragged: short first/last chunks trim fill/drain
CHMAX = max(CHUNKS)
NCHUNK = len(CHUNKS)
C3 = (1.0 / math.sqrt(2.0)) ** 3

_CACHE = {}


def _bmat():
    b = np.zeros((128, 128), np.float32)
    for j in range(64):
        b[2 * j, j] = 1.0
        b[2 * j + 1, j] = 1.0
        b[2 * j, 64 + j] = 1.0
        b[2 * j + 1, 64 + j] = -1.0
    return b


def _build_bass():
    nc = bacc.Bacc()
    x = nc.dram_tensor("x", [NPAIRS, 2, 256, 256], F32, kind="ExternalInput")
    bm = nc.dram_tensor("bmat", [128, 128], F32, kind="ExternalInput")
    y = nc.dram_tensor("y", [2, 128, 2, 2, NPAIRS, 128], F32,
                       kind="ExternalOutput")

    add = mybir.AluOpType.add
    sub = mybir.AluOpType.subtract

    with TileContext(nc) as tc:
        with tc.tile_pool(name="const", bufs=1) as cpool, \
             tc.tile_pool(name="io", bufs=3) as io_pool, \
             tc.tile_pool(name="mid", bufs=3) as mid_pool, \
             tc.tile_pool(name="ps", bufs=1, space="PSUM") as ps_pool:
            B = cpool.tile([128, 128], F32, name="B")
            nc.scalar.dma_start(out=B[:, :], in_=bm[:, :])
            p0 = 0
            for ci, CH in enumerate(CHUNKS):
                X = {}
                for u in range(2):
                    for f in range(2):
                        Xt = io_pool.tile([128, CH, 256], F32, name="X",
                                          tag=f"X{f}{u}", bufs=4,
                                          padded_shape=[128, CHMAX, 256])
                        nc.sync.dma_start(
                            out=Xt[:, :, :],
                            in_=x[p0:p0 + CH, f, 128 * u:128 * (u + 1)]
                                .rearrange("p r w -> r p w"),
                        )
                        X[(f, u)] = Xt
                # F stage: A_u = f0 + f1, D_u = f0 - f1
                AD = {}
                for u in range(2):
                    for t in range(2):       # 0: A (sum), 1: D (diff)
                        M = mid_pool.tile([128, CH, 256], F32, name="M",
                                          tag=f"M{t}{u}",
                                          padded_shape=[128, CHMAX, 256])
                        nc.vector.tensor_tensor(
                            M[:, :, :], X[(0, u)][:, :, :], X[(1, u)][:, :, :],
                            add if t == 0 else sub)
                        AD[(t, u)] = M
                # H stage on PE -> PSUM
                E = {}
                for (t, u), M in AD.items():
                    i = 2 * t + u
                    P = ps_pool.tile([128, CH, 256], F32, name="P", tag=f"P{i}",
                                     padded_shape=[128, CHMAX, 256])
                    Pf = P.rearrange("j p w -> j (p w)")
                    Mf = M.rearrange("j p w -> j (p w)")
                    for n0 in range(0, CH * 256, 512):  # one PSUM bank per matmul
                        n1 = min(n0 + 512, CH * 256)
                        nc.tensor.matmul(
                            Pf[:, n0:n1], B[:, :], Mf[:, n0:n1])
                    # evacuate only the odd columns (ACT): the W-stage
                    # 2-input op may read at most one operand from PSUM
                    Od = mid_pool.tile([128, CH * 128], F32, name="Od",
                                       tag=f"O{i}",
                                       padded_shape=[128, CHMAX * 128])
                    nc.scalar.copy(
                        Od[:, :],
                        P.rearrange("j p (w r) -> j (p w) r", r=2)[:, :, 1])
                    E[(t, u)] = (P, Od)
                # W stage (DVE): even cols from PSUM, odd from SBUF, *C3 fused
                for u in range(2):
                    YU = io_pool.tile([128, 2, 2, CH * 128], F32, name="YU",
                                      tag=f"YU{u}",
                                      padded_shape=[128, 2, 2, CHMAX * 128])
                    for t in range(2):
                        P, Od = E[(t, u)]
                        xe = P.rearrange("j p (w r) -> j (p w) r", r=2)[:, :, 0]
                        xo = Od[:, :]
                        # out = (in0 - in1*s0 - s1) * imm2
                        nc.vector.ln_bwd_dx(YU[:, t, 0, :], xe, xo, -1.0, 0.0, C3)
                        nc.vector.ln_bwd_dx(YU[:, t, 1, :], xe, xo, 1.0, 0.0, C3)
                    nc.scalar.dma_start(
                        out=y[u, :, :, :, p0:p0 + CH]
                            .rearrange("j t e p w -> j t e (p w)"),
                        in_=YU[:, :, :, :],
                    )
                p0 += CH
    nc.compile()
    return nc


def _get_nc():
    if "nc" not in _CACHE:
        _CACHE["nc"] = _build_bass()
    return _CACHE["nc"]


def _shard_inputs(video):
    video = np.ascontiguousarray(np.asarray(video), dtype=np.float32)
    bm = _bmat()
    in_maps = []
    for k in range(NCORES):
        shard = np.ascontiguousarray(
            video[:, :, 2 * k:2 * k + 2]).reshape(NPAIRS, 2, 256, 256)
        in_maps.append({"x": shard, "bmat": bm})
    return in_maps


def _unshard_outputs(results):
    # y[u, j, t, e, p, w]; j = qq*64 + jj; h = u*64 + jj; s = (t, qq, e)
    ys = np.stack([np.asarray(r["y"]) for r in results])  # [8,2,128,2,2,12,128]
    ys = ys.reshape(NCORES, 2, 2, 64, 2, 2, NPAIRS, 128)
    #      dims: (f, u, qq, jj, t, e, p, w)
    ys = ys.transpose(4, 2, 5, 6, 0, 1, 3, 7)
    #      -> (t, qq, e, p, f, u, jj, w)
    ys = ys.reshape(8, 4, 3, NCORES, 128, 128)            # (s, b, c, f, h, w)
    return tuple(np.ascontiguousarray(ys[s]) for s in range(8))


def run(video, **spmd_kwargs):
    nc = _get_nc()
    res = run_bass_kernel_spmd(
        nc, _shard_inputs(video), core_ids=list(range(NCORES)), **spmd_kwargs
    )
    return _unshard_outputs(res.results), res


def kernel(video):
    out, _ = run(video)
    return out
